# revision 8
# baseline (speedup 1.0000x reference)
"""Trainium2 Bass kernel for nn_BinaryQuantumClassifier.

Math: the 4-qubit circuit collapses to a closed form. Per sample, with
theta_j = pi * (x @ W_ctq.T + b_ctq)_j  (j = 4r + i, reuse r, qubit i):
    d_i(theta) = a_i + b_i sin(theta) + c_i cos(theta)
(a/b/c derived from the fixed per-qubit unitary RZ RY RX after RY(theta) H|0>),
and the CNOT chain maps Z-expectations to products of the d_i:
    z0 = d1 d2 d3, z1 = d0 d1, z2 = d0 d1 d2, z3 = d0 d1 d2 d3.
Output = (mean over r of z) @ W_cls.T + b_cls.

Device plan per core (8192 samples, x pre-transposed on host to [512, 8192]):
  - 32 slabs x 256 samples: out[8, 256] = W_chunk.T @ xT_chunk accumulated over
    4 K-chunks on the PE (float32r).
  - ScalarE copies each PSUM slab (+ b_ctq bias) into staging tiles C_g
    [128, 256] at 32-aligned offsets (walrus requires 32-aligned bases).
  - Strided DMAs regroup into E [128, 512]: partition = sample-group,
    free = j*64 + u  (sample n = 64*p + u). All later elementwise ops are
    same-base TT ops (walrus: TT inputs must share base partition).
  - sin/cos with exact range reduction: k2 = ((y + 1.5*2^24) - 1.5*2^24)
    rounds y to the nearest even integer, r = y - k2 in [-1, 1]; ScalarE Sin
    evaluates sin(pi r); cos via one-period wrap of r + 0.5.
  - products/means/final-linear on free-axis slices; one [128, 128] output.
"""

import numpy as np

import concourse.bass as bass
import concourse.mybir as mybir
from concourse import bass_utils
from concourse.tile import TileContext

B, D, NQ = 65536, 512, 4
NCORES = 8
BC = B // NCORES            # 8192 samples per core
NCH = D // 128              # 4 K-chunks
NS = 32                     # slabs per core
F = BC // NS                # 256 samples per slab
FE = BC // 128              # 64 samples per partition in epilogue layout
M2 = float(np.float32(1.5 * 2 ** 24))   # round-to-even-integer magic
PI = float(np.pi)
MM_DT = mybir.dt.float32r   # PE dtype for the skinny matmul
AL = mybir.AluOpType
AF = mybir.ActivationFunctionType
F32 = mybir.dt.float32


def _split_waits(nc, max_waits=1):
    """walrus in this env accepts at most one sync-wait per instruction;
    move extras onto preceding same-engine NoOps."""
    for fn in nc.m.functions:
        for blk in fn.blocks:
            new_list = []
            for inst in blk.instructions:
                si = inst.sync_info
                if si is not None and len(si.on_wait) > max_waits:
                    waits = list(si.on_wait)
                    keep, extra = waits[-max_waits:], waits[:-max_waits]
                    for k, w in enumerate(extra):
                        new_list.append(mybir.InstNoOp(
                            name=f"{inst.name}-ws{k}", engine=inst.engine,
                            ins=[], outs=[],
                            sync_info=mybir.SyncInfo(on_wait=[w], on_update=[])))
                    si.on_wait = keep
                    inst.sync_info = si
                new_list.append(inst)
            blk.instructions = new_list


def _build_nc():
    nc = bass.Bass("TRN2", target_bir_lowering=False)
    xt_d = nc.dram_tensor("xt", [D, BC], MM_DT, kind="ExternalInput").ap()
    wt_d = nc.dram_tensor("wt", [D, 8], MM_DT, kind="ExternalInput").ap()
    cv_d = nc.dram_tensor("cv", [128, 16], F32, kind="ExternalInput").ap()
    cvt_d = nc.dram_tensor("cvt", [128, 3 * 8 * FE], F32, kind="ExternalInput").ap()
    o_d = nc.dram_tensor("o", [128, 2 * FE], F32, kind="ExternalOutput").ap()

    JW = 8 * FE   # 512: width of E

    with TileContext(nc) as tc:
        with tc.tile_pool(name="wp", bufs=1) as wpool, \
             tc.tile_pool(name="xp", bufs=3) as xpool, \
             tc.tile_pool(name="pp", bufs=4, space="PSUM") as pspool, \
             tc.tile_pool(name="cp", bufs=2) as cpool, \
             tc.tile_pool(name="ep", bufs=1) as epool:
            wts = []
            for k in range(NCH):
                wk = wpool.tile([128, 8], MM_DT, name=f"wk{k}")
                nc.sync.dma_start(wk[:], wt_d[128 * k:128 * (k + 1), :])
                wts.append(wk)
            cv = wpool.tile([128, 16], F32)
            nc.sync.dma_start(cv[:], cv_d[:])
            b_band = cv[:, 0:1]
            cvt = wpool.tile([128, 3 * JW], F32)
            nc.sync.dma_start(cvt[:], cvt_d[:])
            aT, bT, cT = cvt[:, 0:JW], cvt[:, JW:2 * JW], cvt[:, 2 * JW:3 * JW]

            # E: partition p = sample-group, free = j*FE + u (n = 64 p + u)
            E = epool.tile([128, JW], F32)
            for g in range(NS // 4):
                C = cpool.tile([128, F], F32, tag="C", name=f"C{g}")
                for q in range(4):
                    s = 4 * g + q
                    ps = pspool.tile([8, F], F32, tag="ps", name=f"ps{s}")
                    for k in range(NCH):
                        xk = xpool.tile([128, F], MM_DT, tag=f"xk{k}", name=f"x_{s}_{k}")
                        nc.sync.dma_start(xk[:], xt_d[128 * k:128 * (k + 1), s * F:(s + 1) * F])
                        nc.tensor.matmul(ps[:, :], wts[k][:], xk[:],
                                         start=(k == 0), stop=(k == NCH - 1))
                    # rows [32q, 32q+8): j = 4r + i, plus b_ctq bias
                    nc.scalar.activation(C[32 * q:32 * q + 8, :], ps[:, :],
                                         AF.Identity, bias=b_band[32 * q:32 * q + 8, :],
                                         scale=1.0)
                # regroup: per (q, tb): C[32q:32q+8, 64tb:64tb+64] -> one E row
                # (dst partition p = 16g + 4q + tb, full 512-wide free row)
                for q in range(4):
                    for tb in range(4):
                        p = 16 * g + 4 * q + tb
                        nc.sync.dma_start(E[p:p + 1, :],
                                          C[32 * q:32 * q + 8, 64 * tb:64 * tb + 64])

            # ---- scalar chain ----
            k2 = epool.tile([128, JW], F32)
            r_ = epool.tile([128, JW], F32)
            r2 = epool.tile([128, JW], F32)
            r3 = epool.tile([128, JW], F32)
            s_ = epool.tile([128, JW], F32)
            cc = epool.tile([128, JW], F32)
            t1 = epool.tile([128, JW], F32)
            t2 = epool.tile([128, JW], F32)
            d_ = epool.tile([128, JW], F32)
            nc.vector.tensor_scalar(k2[:], E[:], M2, M2, AL.add, AL.subtract)
            nc.vector.tensor_sub(r_[:], E[:], k2[:])           # y mod 2 -> [-1, 1]
            nc.scalar.activation(s_[:], r_[:], AF.Sin, scale=PI)
            # cos(pi y) = cos(pi |r|) = sin(pi (0.5 - |r|)), arg in [-pi/2, pi/2]
            nc.vector.tensor_scalar(r3[:], r_[:], -1.0, None, AL.mult)
            nc.vector.tensor_max(r2[:], r_[:], r3[:])          # |r|
            nc.vector.tensor_scalar(r2[:], r2[:], -1.0, 0.5, AL.mult, AL.add)
            nc.scalar.activation(cc[:], r2[:], AF.Sin, scale=PI)
            nc.vector.tensor_mul(t1[:], s_[:], bT)             # b_i sin
            nc.vector.tensor_mul(t2[:], cc[:], cT)             # c_i cos
            nc.vector.tensor_add(t1[:], t1[:], t2[:])
            nc.vector.tensor_add(d_[:], t1[:], aT)             # d = a + b sin + c cos

            def dj(r, i):
                j = 4 * r + i
                return d_[:, j * FE:(j + 1) * FE]

            # ---- products: z_k^r at Z[:, (2k + r)*FE] ----
            S_ = epool.tile([128, 2 * FE], F32)
            Z_ = epool.tile([128, 8 * FE], F32)

            def zs(k, r):
                return Z_[:, (2 * k + r) * FE:(2 * k + r + 1) * FE]

            for r in range(2):
                u_ = S_[:, r * FE:(r + 1) * FE]
                nc.vector.tensor_mul(u_, dj(r, 1), dj(r, 2))        # d1 d2
                nc.vector.tensor_mul(zs(1, r), dj(r, 0), dj(r, 1))  # z1
                nc.vector.tensor_mul(zs(2, r), dj(r, 0), u_)        # z2
                nc.vector.tensor_mul(zs(0, r), u_, dj(r, 3))        # z0
                nc.vector.tensor_mul(zs(3, r), zs(2, r), dj(r, 3))  # z3
            Mn = epool.tile([128, 4 * FE], F32)
            for k in range(4):
                nc.vector.tensor_add(Mn[:, k * FE:(k + 1) * FE], zs(k, 0), zs(k, 1))

            # ---- final linear: W' = 0.5*W_cls via cv columns ----
            O1 = epool.tile([128, 2 * FE], F32)
            O2 = epool.tile([128, 2 * FE], F32)

            def mk(k):
                return Mn[:, k * FE:(k + 1) * FE]

            for c in range(2):
                o1 = O1[:, c * FE:(c + 1) * FE]
                o2 = O2[:, c * FE:(c + 1) * FE]
                nc.vector.tensor_scalar(o1, mk(0), cv[:, 4 + 4 * c:5 + 4 * c],
                                        cv[:, 12 + c:13 + c], AL.mult, AL.add)
                nc.vector.scalar_tensor_tensor(o2, mk(1), cv[:, 5 + 4 * c:6 + 4 * c],
                                               o1, AL.mult, AL.add)
                nc.vector.scalar_tensor_tensor(o1, mk(2), cv[:, 6 + 4 * c:7 + 4 * c],
                                               o2, AL.mult, AL.add)
                nc.vector.scalar_tensor_tensor(o2, mk(3), cv[:, 7 + 4 * c:8 + 4 * c],
                                               o1, AL.mult, AL.add)
            nc.sync.dma_start(o_d[:], O2[:])

    return nc


_NC_CACHE = {}


def _get_nc(split=True):
    key = ("nc", split)
    if key not in _NC_CACHE:
        nc = _build_nc()
        if split:
            _split_waits(nc)
        _NC_CACHE[key] = nc
    return _NC_CACHE[key]


def _qubit_abc(q_params):
    """Exact (a_i, b_i, c_i) with d_i(theta) = a + b sin(theta) + c cos(theta)."""
    out = np.zeros((NQ, 3), np.float64)
    for i in range(NQ):
        pa, pb, pc = [float(v) for v in q_params[3 * i:3 * i + 3]]

        def rx(t):
            return np.array([[np.cos(t / 2), -1j * np.sin(t / 2)],
                             [-1j * np.sin(t / 2), np.cos(t / 2)]])

        def ry(t):
            return np.array([[np.cos(t / 2), -np.sin(t / 2)],
                             [np.sin(t / 2), np.cos(t / 2)]])

        def rz(t):
            return np.array([[np.exp(-0.5j * t), 0], [0, np.exp(0.5j * t)]])

        H = np.array([[1, 1], [1, -1]]) / np.sqrt(2)
        U = rz(pc) @ ry(pb) @ rx(pa)

        def dfun(theta):
            v = U @ ry(theta) @ H @ np.array([1.0, 0.0])
            pr = np.abs(v) ** 2
            return pr[0] - pr[1]

        d0, dpi, dh = dfun(0.0), dfun(np.pi), dfun(np.pi / 2)
        a = (d0 + dpi) / 2
        c = (d0 - dpi) / 2
        b = dh - a
        out[i] = (a, b, c)
    return out


def _make_consts(b_ctq, q_params, W_cls, b_cls):
    abc = _qubit_abc(q_params)
    cv = np.zeros((128, 16), np.float32)
    for p in range(128):
        cv[p, 0] = b_ctq[p % 32] if (p % 32) < 8 else 0.0
    wp = 0.5 * np.asarray(W_cls, np.float64)      # mean over r folded in
    for c in range(2):
        for k in range(4):
            cv[:, 4 + 4 * c + k] = np.float32(wp[c, k])
        cv[:, 12 + c] = np.float32(b_cls[c])
    JW = 8 * FE
    cvt = np.zeros((128, 3 * JW), np.float32)
    for j in range(8):
        i = j % 4
        cvt[:, 0 * JW + j * FE:0 * JW + (j + 1) * FE] = np.float32(abc[i, 0])
        cvt[:, 1 * JW + j * FE:1 * JW + (j + 1) * FE] = np.float32(abc[i, 1])
        cvt[:, 2 * JW + j * FE:2 * JW + (j + 1) * FE] = np.float32(abc[i, 2])
    return cv, cvt


def make_in_maps(x, W_ctq, b_ctq, q_params, W_cls, b_cls):
    wt = np.ascontiguousarray(np.asarray(W_ctq, np.float32).T)  # [512, 8]
    cv, cvt = _make_consts(np.asarray(b_ctq, np.float32),
                           np.asarray(q_params, np.float32),
                           np.asarray(W_cls, np.float32),
                           np.asarray(b_cls, np.float32))
    x = np.asarray(x, np.float32)
    in_maps = []
    for c in range(NCORES):
        xt = np.ascontiguousarray(x[c * BC:(c + 1) * BC].T)     # [512, 8192]
        in_maps.append({"xt": xt, "wt": wt, "cv": cv, "cvt": cvt})
    return in_maps


def assemble_output(results):
    out = np.empty((B, 2), np.float32)
    for core in range(NCORES):
        o = results[core]["o"]                                   # [128, 2*FE]
        for c in range(2):
            out[core * BC:(core + 1) * BC, c] = o[:, c * FE:(c + 1) * FE].reshape(BC)
    return out


def kernel(x, W_ctq, b_ctq, q_params, W_cls, b_cls):
    nc = _get_nc()
    in_maps = make_in_maps(x, W_ctq, b_ctq, q_params, W_cls, b_cls)
    res = bass_utils.run_bass_kernel_spmd(nc, in_maps, core_ids=list(range(NCORES)))
    return assemble_output(res.results)


# revision 10
# speedup vs baseline: 1.5991x; 1.5991x over previous
"""Trainium2 Bass kernel for nn_BinaryQuantumClassifier.

Math: the 4-qubit circuit collapses to a closed form. Per sample, with
theta_j = pi * (x @ W_ctq.T + b_ctq)_j  (j = 4r + i, reuse r, qubit i):
    d_i(theta) = a_i + b_i sin(theta) + c_i cos(theta)
(a/b/c derived from the fixed per-qubit unitary RZ RY RX after RY(theta) H|0>),
and the CNOT chain maps Z-expectations to products of the d_i:
    z0 = d1 d2 d3, z1 = d0 d1, z2 = d0 d1 d2, z3 = d0 d1 d2 d3.
Output = (mean over r of z) @ W_cls.T + b_cls.

Device plan per core (8192 samples). x is relayouted on the host so that the
PE can use it as the STATIONARY operand: lhsT = x-chunk [128 D x 128 samples],
rhs = W-chunk [128 D x 8]; out = [128 samples, 8] accumulates over 4 K-chunks
in PSUM. 64 sample-groups; 8 groups share one PSUM bank tile [128, 64].
One ScalarE copy per bank (strided free AP) assembles E [128, 512] with
free = j*64 + u (j = 4r + i, u = sample-group; sample n = 128*u + p).
Epilogue (all elementwise, same-partition-base, free-axis slicing):
  - yb = E + b_ctq (free-axis const tile)
  - sin/cos with exact range reduction: k2 = ((y + 1.5*2^24) - 1.5*2^24)
    rounds y to the nearest even integer, r = y - k2 in [-1, 1]; ScalarE Sin
    evaluates sin(pi r); cos = sin(pi (0.5 - |r|)).
  - d = a + b sin + c cos (free-axis const tiles), CNOT products, mean over r,
    final 4->2 linear via per-partition scalar chains; one [128, 128] output.
"""

import numpy as np

import concourse.bass as bass
import concourse.mybir as mybir
from concourse import bass_utils
from concourse.tile import TileContext

B, D, NQ = 65536, 512, 4
NCORES = 8
BC = B // NCORES            # 8192 samples per core
NCH = D // 128              # 4 K-chunks
NS = 32                     # slabs per core
F = BC // NS                # 256 samples per slab
FE = BC // 128              # 64 samples per partition in epilogue layout
M2 = float(np.float32(1.5 * 2 ** 24))   # round-to-even-integer magic
PI = float(np.pi)
MM_DT = mybir.dt.float32    # full-precision PE matmul (moving dim is only 8)
NG = BC // 128              # 64 sample-groups per core
GPB = 8                     # groups per PSUM bank tile
AL = mybir.AluOpType
AF = mybir.ActivationFunctionType
F32 = mybir.dt.float32


def _split_waits(nc, max_waits=1):
    """walrus in this env accepts at most one sync-wait per instruction;
    move extras onto preceding same-engine NoOps."""
    for fn in nc.m.functions:
        for blk in fn.blocks:
            new_list = []
            for inst in blk.instructions:
                si = inst.sync_info
                if si is not None and len(si.on_wait) > max_waits:
                    waits = list(si.on_wait)
                    keep, extra = waits[-max_waits:], waits[:-max_waits]
                    for k, w in enumerate(extra):
                        new_list.append(mybir.InstNoOp(
                            name=f"{inst.name}-ws{k}", engine=inst.engine,
                            ins=[], outs=[],
                            sync_info=mybir.SyncInfo(on_wait=[w], on_update=[])))
                    si.on_wait = keep
                    inst.sync_info = si
                new_list.append(inst)
            blk.instructions = new_list


def _build_nc():
    nc = bass.Bass("TRN2", target_bir_lowering=False)
    # xt3[p, m*512 + k*128 + ms] = x[128*m + ms, 128*k + p]
    xt_d = nc.dram_tensor("xt", [128, BC * NCH], MM_DT, kind="ExternalInput").ap()
    wt_d = nc.dram_tensor("wt", [D, 8], MM_DT, kind="ExternalInput").ap()
    cv_d = nc.dram_tensor("cv", [128, 16], F32, kind="ExternalInput").ap()
    # cvt: [bB | aT | bT | cT], each [128, 512] broadcast along partitions
    cvt_d = nc.dram_tensor("cvt", [128, 4 * 8 * FE], F32, kind="ExternalInput").ap()
    o_d = nc.dram_tensor("o", [128, 2 * FE], F32, kind="ExternalOutput").ap()

    JW = 8 * FE   # 512: width of E

    with TileContext(nc) as tc:
        with tc.tile_pool(name="wp", bufs=1) as wpool, \
             tc.tile_pool(name="xp", bufs=2) as xpool, \
             tc.tile_pool(name="pp", bufs=4, space="PSUM") as pspool, \
             tc.tile_pool(name="ep", bufs=1) as epool:
            wts = []
            for k in range(NCH):
                wk = wpool.tile([128, 8], MM_DT, name=f"wk{k}")
                nc.sync.dma_start(wk[:], wt_d[128 * k:128 * (k + 1), :])
                wts.append(wk)
            cv = wpool.tile([128, 16], F32)
            nc.sync.dma_start(cv[:], cv_d[:])
            cvt = wpool.tile([128, 4 * JW], F32)
            nc.sync.dma_start(cvt[:], cvt_d[:])
            bB = cvt[:, 0:JW]
            aT, bT, cT = cvt[:, JW:2 * JW], cvt[:, 2 * JW:3 * JW], cvt[:, 3 * JW:4 * JW]

            # E: partition p = sample-in-group, free = j*FE + u (n = 128 u + p)
            E = epool.tile([128, JW], F32)
            GL = GPB * NCH * 128          # 4096: L-tile free width (8 groups)
            for g in range(NG // GPB):    # 8 bank-tiles
                L = xpool.tile([128, GL], MM_DT, tag="L", name=f"L{g}")
                nc.sync.dma_start(L[:], xt_d[:, g * GL:(g + 1) * GL])
                ps = pspool.tile([128, GPB * 8], F32, tag="ps", name=f"ps{g}")
                for mm in range(GPB):
                    for k in range(NCH):
                        off = mm * (NCH * 128) + k * 128
                        nc.tensor.matmul(ps[:, 8 * mm:8 * mm + 8],
                                         L[:, off:off + 128], wts[k][:],
                                         start=(k == 0), stop=(k == NCH - 1))
                # assemble E[:, j*FE + 8g + m] = ps[:, 8m + j]
                in_ap = ps.rearrange("p (m j) -> p j m", j=8)
                out_ap = E.rearrange("p (j u) -> p j u", j=8)[:, :, GPB * g:GPB * (g + 1)]
                nc.scalar.copy(out_ap, in_ap)

            # ---- scalar chain ----
            yb = epool.tile([128, JW], F32)
            k2 = epool.tile([128, JW], F32)
            r_ = epool.tile([128, JW], F32)
            r2 = epool.tile([128, JW], F32)
            r3 = epool.tile([128, JW], F32)
            s_ = epool.tile([128, JW], F32)
            cc = epool.tile([128, JW], F32)
            t1 = epool.tile([128, JW], F32)
            t2 = epool.tile([128, JW], F32)
            d_ = epool.tile([128, JW], F32)
            nc.vector.tensor_add(yb[:], E[:], bB)              # + b_ctq
            nc.vector.tensor_scalar(k2[:], yb[:], M2, M2, AL.add, AL.subtract)
            nc.vector.tensor_sub(r_[:], yb[:], k2[:])          # y mod 2 -> [-1, 1]
            nc.scalar.activation(s_[:], r_[:], AF.Sin, scale=PI)
            # cos(pi y) = cos(pi |r|) = sin(pi (0.5 - |r|)), arg in [-pi/2, pi/2]
            nc.vector.tensor_scalar(r3[:], r_[:], -1.0, None, AL.mult)
            nc.vector.tensor_max(r2[:], r_[:], r3[:])          # |r|
            nc.vector.tensor_scalar(r2[:], r2[:], -1.0, 0.5, AL.mult, AL.add)
            nc.scalar.activation(cc[:], r2[:], AF.Sin, scale=PI)
            nc.vector.tensor_mul(t1[:], s_[:], bT)             # b_i sin
            nc.vector.tensor_mul(t2[:], cc[:], cT)             # c_i cos
            nc.vector.tensor_add(t1[:], t1[:], t2[:])
            nc.vector.tensor_add(d_[:], t1[:], aT)             # d = a + b sin + c cos

            def dj(r, i):
                j = 4 * r + i
                return d_[:, j * FE:(j + 1) * FE]

            # ---- products: z_k^r at Z[:, (2k + r)*FE] ----
            S_ = epool.tile([128, 2 * FE], F32)
            Z_ = epool.tile([128, 8 * FE], F32)

            def zs(k, r):
                return Z_[:, (2 * k + r) * FE:(2 * k + r + 1) * FE]

            for r in range(2):
                u_ = S_[:, r * FE:(r + 1) * FE]
                nc.vector.tensor_mul(u_, dj(r, 1), dj(r, 2))        # d1 d2
                nc.vector.tensor_mul(zs(1, r), dj(r, 0), dj(r, 1))  # z1
                nc.vector.tensor_mul(zs(2, r), dj(r, 0), u_)        # z2
                nc.vector.tensor_mul(zs(0, r), u_, dj(r, 3))        # z0
                nc.vector.tensor_mul(zs(3, r), zs(2, r), dj(r, 3))  # z3
            Mn = epool.tile([128, 4 * FE], F32)
            for k in range(4):
                nc.vector.tensor_add(Mn[:, k * FE:(k + 1) * FE], zs(k, 0), zs(k, 1))

            # ---- final linear: W' = 0.5*W_cls via cv columns ----
            O1 = epool.tile([128, 2 * FE], F32)
            O2 = epool.tile([128, 2 * FE], F32)

            def mk(k):
                return Mn[:, k * FE:(k + 1) * FE]

            for c in range(2):
                o1 = O1[:, c * FE:(c + 1) * FE]
                o2 = O2[:, c * FE:(c + 1) * FE]
                nc.vector.tensor_scalar(o1, mk(0), cv[:, 4 + 4 * c:5 + 4 * c],
                                        cv[:, 12 + c:13 + c], AL.mult, AL.add)
                nc.vector.scalar_tensor_tensor(o2, mk(1), cv[:, 5 + 4 * c:6 + 4 * c],
                                               o1, AL.mult, AL.add)
                nc.vector.scalar_tensor_tensor(o1, mk(2), cv[:, 6 + 4 * c:7 + 4 * c],
                                               o2, AL.mult, AL.add)
                nc.vector.scalar_tensor_tensor(o2, mk(3), cv[:, 7 + 4 * c:8 + 4 * c],
                                               o1, AL.mult, AL.add)
            nc.sync.dma_start(o_d[:], O2[:])

    return nc


_NC_CACHE = {}


def _get_nc(split=True):
    key = ("nc", split)
    if key not in _NC_CACHE:
        nc = _build_nc()
        if split:
            _split_waits(nc)
        _NC_CACHE[key] = nc
    return _NC_CACHE[key]


def _qubit_abc(q_params):
    """Exact (a_i, b_i, c_i) with d_i(theta) = a + b sin(theta) + c cos(theta)."""
    out = np.zeros((NQ, 3), np.float64)
    for i in range(NQ):
        pa, pb, pc = [float(v) for v in q_params[3 * i:3 * i + 3]]

        def rx(t):
            return np.array([[np.cos(t / 2), -1j * np.sin(t / 2)],
                             [-1j * np.sin(t / 2), np.cos(t / 2)]])

        def ry(t):
            return np.array([[np.cos(t / 2), -np.sin(t / 2)],
                             [np.sin(t / 2), np.cos(t / 2)]])

        def rz(t):
            return np.array([[np.exp(-0.5j * t), 0], [0, np.exp(0.5j * t)]])

        H = np.array([[1, 1], [1, -1]]) / np.sqrt(2)
        U = rz(pc) @ ry(pb) @ rx(pa)

        def dfun(theta):
            v = U @ ry(theta) @ H @ np.array([1.0, 0.0])
            pr = np.abs(v) ** 2
            return pr[0] - pr[1]

        d0, dpi, dh = dfun(0.0), dfun(np.pi), dfun(np.pi / 2)
        a = (d0 + dpi) / 2
        c = (d0 - dpi) / 2
        b = dh - a
        out[i] = (a, b, c)
    return out


def _make_consts(b_ctq, q_params, W_cls, b_cls):
    abc = _qubit_abc(q_params)
    cv = np.zeros((128, 16), np.float32)
    wp = 0.5 * np.asarray(W_cls, np.float64)      # mean over r folded in
    for c in range(2):
        for k in range(4):
            cv[:, 4 + 4 * c + k] = np.float32(wp[c, k])
        cv[:, 12 + c] = np.float32(b_cls[c])
    JW = 8 * FE
    cvt = np.zeros((128, 4 * JW), np.float32)
    for j in range(8):
        i = j % 4
        cvt[:, 0 * JW + j * FE:0 * JW + (j + 1) * FE] = np.float32(b_ctq[j])
        cvt[:, 1 * JW + j * FE:1 * JW + (j + 1) * FE] = np.float32(abc[i, 0])
        cvt[:, 2 * JW + j * FE:2 * JW + (j + 1) * FE] = np.float32(abc[i, 1])
        cvt[:, 3 * JW + j * FE:3 * JW + (j + 1) * FE] = np.float32(abc[i, 2])
    return cv, cvt


def make_in_maps(x, W_ctq, b_ctq, q_params, W_cls, b_cls):
    wt = np.ascontiguousarray(np.asarray(W_ctq, np.float32).T)  # [512, 8]
    cv, cvt = _make_consts(np.asarray(b_ctq, np.float32),
                           np.asarray(q_params, np.float32),
                           np.asarray(W_cls, np.float32),
                           np.asarray(b_cls, np.float32))
    x = np.asarray(x, np.float32)
    in_maps = []
    for c in range(NCORES):
        xs = x[c * BC:(c + 1) * BC]                             # [8192, 512]
        # xt3[p, m*512 + k*128 + ms] = xs[128 m + ms, 128 k + p]
        xt = np.ascontiguousarray(
            xs.reshape(NG, 128, NCH, 128).transpose(3, 0, 2, 1).reshape(128, BC * NCH))
        in_maps.append({"xt": xt, "wt": wt, "cv": cv, "cvt": cvt})
    return in_maps


def assemble_output(results):
    out = np.empty((B, 2), np.float32)
    for core in range(NCORES):
        o = results[core]["o"]                                   # [128, 2*FE]
        for c in range(2):
            # o[p, c*FE + u] = out_c(sample 128 u + p)
            out[core * BC:(core + 1) * BC, c] = \
                o[:, c * FE:(c + 1) * FE].T.reshape(BC)
    return out


def kernel(x, W_ctq, b_ctq, q_params, W_cls, b_cls):
    nc = _get_nc()
    in_maps = make_in_maps(x, W_ctq, b_ctq, q_params, W_cls, b_cls)
    res = bass_utils.run_bass_kernel_spmd(nc, in_maps, core_ids=list(range(NCORES)))
    return assemble_output(res.results)


# revision 11
# speedup vs baseline: 2.6418x; 1.6520x over previous
"""Trainium2 Bass kernel for nn_BinaryQuantumClassifier.

Math: the 4-qubit circuit collapses to a closed form. Per sample, with
theta_j = pi * (x @ W_ctq.T + b_ctq)_j  (j = 4r + i, reuse r, qubit i):
    d_i(theta) = a_i + b_i sin(theta) + c_i cos(theta)
(a/b/c derived from the fixed per-qubit unitary RZ RY RX after RY(theta) H|0>),
and the CNOT chain maps Z-expectations to products of the d_i:
    z0 = d1 d2 d3, z1 = d0 d1, z2 = d0 d1 d2, z3 = d0 d1 d2 d3.
Output = (mean over r of z) @ W_cls.T + b_cls.

Device plan per core (8192 samples). x is relayouted on the host so that the
PE can use it as the STATIONARY operand: lhsT = x-chunk [128 D x 128 samples],
rhs = W-chunk [128 D x 8]; out = [128 samples, 8] accumulates over 4 K-chunks
in PSUM. 64 sample-groups; 8 groups share one PSUM bank tile [128, 64].
One ScalarE copy per bank (strided free AP) assembles E [128, 512] with
free = j*64 + u (j = 4r + i, u = sample-group; sample n = 128*u + p).
Epilogue (all elementwise, same-partition-base, free-axis slicing):
  - yb = E + b_ctq (free-axis const tile)
  - sin/cos with exact range reduction: k2 = ((y + 1.5*2^24) - 1.5*2^24)
    rounds y to the nearest even integer, r = y - k2 in [-1, 1]; ScalarE Sin
    evaluates sin(pi r); cos = sin(pi (0.5 - |r|)).
  - d = a + b sin + c cos (free-axis const tiles), CNOT products, mean over r,
    final 4->2 linear via per-partition scalar chains; one [128, 128] output.
"""

import numpy as np

import concourse.bass as bass
import concourse.mybir as mybir
from concourse import bass_utils
from concourse.tile import TileContext

B, D, NQ = 65536, 512, 4
NCORES = 8
BC = B // NCORES            # 8192 samples per core
NCH = D // 128              # 4 K-chunks
NS = 32                     # slabs per core
F = BC // NS                # 256 samples per slab
FE = BC // 128              # 64 samples per partition in epilogue layout
M2 = float(np.float32(1.5 * 2 ** 24))   # round-to-even-integer magic
PI = float(np.pi)
MM_DT = mybir.dt.bfloat16   # PE operand dtype: x/W split into bf16 hi+lo
                            # (3 passes hi*hi + hi*lo + lo*hi, fp32 PSUM accum)
NG = BC // 128              # 64 sample-groups per core
GPB = 8                     # groups per PSUM bank tile
AL = mybir.AluOpType
AF = mybir.ActivationFunctionType
F32 = mybir.dt.float32


def _split_waits(nc, max_waits=1):
    """walrus in this env accepts at most one sync-wait per instruction;
    move extras onto preceding same-engine NoOps."""
    for fn in nc.m.functions:
        for blk in fn.blocks:
            new_list = []
            for inst in blk.instructions:
                si = inst.sync_info
                if si is not None and len(si.on_wait) > max_waits:
                    waits = list(si.on_wait)
                    keep, extra = waits[-max_waits:], waits[:-max_waits]
                    for k, w in enumerate(extra):
                        new_list.append(mybir.InstNoOp(
                            name=f"{inst.name}-ws{k}", engine=inst.engine,
                            ins=[], outs=[],
                            sync_info=mybir.SyncInfo(on_wait=[w], on_update=[])))
                    si.on_wait = keep
                    inst.sync_info = si
                new_list.append(inst)
            blk.instructions = new_list


def _build_nc():
    nc = bass.Bass("TRN2", target_bir_lowering=False)
    # x relayout: [p, m*512 + k*128 + ms] = x[128*m + ms, 128*k + p]
    xhi_d = nc.dram_tensor("xhi", [128, BC * NCH], MM_DT, kind="ExternalInput").ap()
    xlo_d = nc.dram_tensor("xlo", [128, BC * NCH], MM_DT, kind="ExternalInput").ap()
    whi_d = nc.dram_tensor("whi", [D, 8], MM_DT, kind="ExternalInput").ap()
    wlo_d = nc.dram_tensor("wlo", [D, 8], MM_DT, kind="ExternalInput").ap()
    cv_d = nc.dram_tensor("cv", [128, 16], F32, kind="ExternalInput").ap()
    # cvt: [bB | aT | bT | cT], each [128, 512] broadcast along partitions
    cvt_d = nc.dram_tensor("cvt", [128, 4 * 8 * FE], F32, kind="ExternalInput").ap()
    o_d = nc.dram_tensor("o", [128, 2 * FE], F32, kind="ExternalOutput").ap()

    JW = 8 * FE   # 512: width of E

    with TileContext(nc) as tc:
        with tc.tile_pool(name="wp", bufs=1) as wpool, \
             tc.tile_pool(name="xp", bufs=2) as xpool, \
             tc.tile_pool(name="pp", bufs=4, space="PSUM") as pspool, \
             tc.tile_pool(name="ep", bufs=1) as epool:
            whis, wlos = [], []
            for k in range(NCH):
                whik = wpool.tile([128, 8], MM_DT, name=f"whik{k}")
                nc.sync.dma_start(whik[:], whi_d[128 * k:128 * (k + 1), :])
                whis.append(whik)
                wlok = wpool.tile([128, 8], MM_DT, name=f"wlok{k}")
                nc.sync.dma_start(wlok[:], wlo_d[128 * k:128 * (k + 1), :])
                wlos.append(wlok)
            cv = wpool.tile([128, 16], F32)
            nc.sync.dma_start(cv[:], cv_d[:])
            cvt = wpool.tile([128, 4 * JW], F32)
            nc.sync.dma_start(cvt[:], cvt_d[:])
            bB = cvt[:, 0:JW]
            aT, bT, cT = cvt[:, JW:2 * JW], cvt[:, 2 * JW:3 * JW], cvt[:, 3 * JW:4 * JW]

            # E: partition p = sample-in-group, free = j*FE + u (n = 128 u + p)
            E = epool.tile([128, JW], F32)
            GL = GPB * NCH * 128          # 4096: L-tile free width (8 groups)
            for g in range(NG // GPB):    # 8 bank-tiles
                Lhi = xpool.tile([128, GL], MM_DT, tag="Lhi", name=f"Lhi{g}")
                nc.sync.dma_start(Lhi[:], xhi_d[:, g * GL:(g + 1) * GL])
                Llo = xpool.tile([128, GL], MM_DT, tag="Llo", name=f"Llo{g}")
                nc.sync.dma_start(Llo[:], xlo_d[:, g * GL:(g + 1) * GL])
                ps = pspool.tile([128, GPB * 8], F32, tag="ps", name=f"ps{g}")
                for mm in range(GPB):
                    for k in range(NCH):
                        off = mm * (NCH * 128) + k * 128
                        out_sl = ps[:, 8 * mm:8 * mm + 8]
                        nc.tensor.matmul(out_sl, Lhi[:, off:off + 128], whis[k][:],
                                         start=(k == 0), stop=False)
                        nc.tensor.matmul(out_sl, Lhi[:, off:off + 128], wlos[k][:],
                                         start=False, stop=False)
                        nc.tensor.matmul(out_sl, Llo[:, off:off + 128], whis[k][:],
                                         start=False, stop=(k == NCH - 1))
                # assemble E[:, j*FE + 8g + m] = ps[:, 8m + j]
                in_ap = ps.rearrange("p (m j) -> p j m", j=8)
                out_ap = E.rearrange("p (j u) -> p j u", j=8)[:, :, GPB * g:GPB * (g + 1)]
                nc.scalar.copy(out_ap, in_ap)

            # ---- scalar chain ----
            yb = epool.tile([128, JW], F32)
            k2 = epool.tile([128, JW], F32)
            r_ = epool.tile([128, JW], F32)
            r2 = epool.tile([128, JW], F32)
            r3 = epool.tile([128, JW], F32)
            s_ = epool.tile([128, JW], F32)
            cc = epool.tile([128, JW], F32)
            t1 = epool.tile([128, JW], F32)
            t2 = epool.tile([128, JW], F32)
            d_ = epool.tile([128, JW], F32)
            nc.vector.tensor_add(yb[:], E[:], bB)              # + b_ctq
            nc.vector.tensor_scalar(k2[:], yb[:], M2, M2, AL.add, AL.subtract)
            nc.vector.tensor_sub(r_[:], yb[:], k2[:])          # y mod 2 -> [-1, 1]
            nc.scalar.activation(s_[:], r_[:], AF.Sin, scale=PI)
            # cos(pi y) = cos(pi |r|) = sin(pi (0.5 - |r|)), arg in [-pi/2, pi/2]
            nc.vector.tensor_scalar(r3[:], r_[:], -1.0, None, AL.mult)
            nc.vector.tensor_max(r2[:], r_[:], r3[:])          # |r|
            nc.vector.tensor_scalar(r2[:], r2[:], -1.0, 0.5, AL.mult, AL.add)
            nc.scalar.activation(cc[:], r2[:], AF.Sin, scale=PI)
            nc.vector.tensor_mul(t1[:], s_[:], bT)             # b_i sin
            nc.vector.tensor_mul(t2[:], cc[:], cT)             # c_i cos
            nc.vector.tensor_add(t1[:], t1[:], t2[:])
            nc.vector.tensor_add(d_[:], t1[:], aT)             # d = a + b sin + c cos

            def dj(r, i):
                j = 4 * r + i
                return d_[:, j * FE:(j + 1) * FE]

            # ---- products: z_k^r at Z[:, (2k + r)*FE] ----
            S_ = epool.tile([128, 2 * FE], F32)
            Z_ = epool.tile([128, 8 * FE], F32)

            def zs(k, r):
                return Z_[:, (2 * k + r) * FE:(2 * k + r + 1) * FE]

            for r in range(2):
                u_ = S_[:, r * FE:(r + 1) * FE]
                nc.vector.tensor_mul(u_, dj(r, 1), dj(r, 2))        # d1 d2
                nc.vector.tensor_mul(zs(1, r), dj(r, 0), dj(r, 1))  # z1
                nc.vector.tensor_mul(zs(2, r), dj(r, 0), u_)        # z2
                nc.vector.tensor_mul(zs(0, r), u_, dj(r, 3))        # z0
                nc.vector.tensor_mul(zs(3, r), zs(2, r), dj(r, 3))  # z3
            Mn = epool.tile([128, 4 * FE], F32)
            for k in range(4):
                nc.vector.tensor_add(Mn[:, k * FE:(k + 1) * FE], zs(k, 0), zs(k, 1))

            # ---- final linear: W' = 0.5*W_cls via cv columns ----
            O1 = epool.tile([128, 2 * FE], F32)
            O2 = epool.tile([128, 2 * FE], F32)

            def mk(k):
                return Mn[:, k * FE:(k + 1) * FE]

            for c in range(2):
                o1 = O1[:, c * FE:(c + 1) * FE]
                o2 = O2[:, c * FE:(c + 1) * FE]
                nc.vector.tensor_scalar(o1, mk(0), cv[:, 4 + 4 * c:5 + 4 * c],
                                        cv[:, 12 + c:13 + c], AL.mult, AL.add)
                nc.vector.scalar_tensor_tensor(o2, mk(1), cv[:, 5 + 4 * c:6 + 4 * c],
                                               o1, AL.mult, AL.add)
                nc.vector.scalar_tensor_tensor(o1, mk(2), cv[:, 6 + 4 * c:7 + 4 * c],
                                               o2, AL.mult, AL.add)
                nc.vector.scalar_tensor_tensor(o2, mk(3), cv[:, 7 + 4 * c:8 + 4 * c],
                                               o1, AL.mult, AL.add)
            nc.sync.dma_start(o_d[:], O2[:])

    return nc


_NC_CACHE = {}


def _get_nc(split=True):
    key = ("nc", split)
    if key not in _NC_CACHE:
        nc = _build_nc()
        if split:
            _split_waits(nc)
        _NC_CACHE[key] = nc
    return _NC_CACHE[key]


def _qubit_abc(q_params):
    """Exact (a_i, b_i, c_i) with d_i(theta) = a + b sin(theta) + c cos(theta)."""
    out = np.zeros((NQ, 3), np.float64)
    for i in range(NQ):
        pa, pb, pc = [float(v) for v in q_params[3 * i:3 * i + 3]]

        def rx(t):
            return np.array([[np.cos(t / 2), -1j * np.sin(t / 2)],
                             [-1j * np.sin(t / 2), np.cos(t / 2)]])

        def ry(t):
            return np.array([[np.cos(t / 2), -np.sin(t / 2)],
                             [np.sin(t / 2), np.cos(t / 2)]])

        def rz(t):
            return np.array([[np.exp(-0.5j * t), 0], [0, np.exp(0.5j * t)]])

        H = np.array([[1, 1], [1, -1]]) / np.sqrt(2)
        U = rz(pc) @ ry(pb) @ rx(pa)

        def dfun(theta):
            v = U @ ry(theta) @ H @ np.array([1.0, 0.0])
            pr = np.abs(v) ** 2
            return pr[0] - pr[1]

        d0, dpi, dh = dfun(0.0), dfun(np.pi), dfun(np.pi / 2)
        a = (d0 + dpi) / 2
        c = (d0 - dpi) / 2
        b = dh - a
        out[i] = (a, b, c)
    return out


def _make_consts(b_ctq, q_params, W_cls, b_cls):
    abc = _qubit_abc(q_params)
    cv = np.zeros((128, 16), np.float32)
    wp = 0.5 * np.asarray(W_cls, np.float64)      # mean over r folded in
    for c in range(2):
        for k in range(4):
            cv[:, 4 + 4 * c + k] = np.float32(wp[c, k])
        cv[:, 12 + c] = np.float32(b_cls[c])
    JW = 8 * FE
    cvt = np.zeros((128, 4 * JW), np.float32)
    for j in range(8):
        i = j % 4
        cvt[:, 0 * JW + j * FE:0 * JW + (j + 1) * FE] = np.float32(b_ctq[j])
        cvt[:, 1 * JW + j * FE:1 * JW + (j + 1) * FE] = np.float32(abc[i, 0])
        cvt[:, 2 * JW + j * FE:2 * JW + (j + 1) * FE] = np.float32(abc[i, 1])
        cvt[:, 3 * JW + j * FE:3 * JW + (j + 1) * FE] = np.float32(abc[i, 2])
    return cv, cvt


def make_in_maps(x, W_ctq, b_ctq, q_params, W_cls, b_cls):
    import ml_dtypes
    bf16 = ml_dtypes.bfloat16
    wt = np.asarray(W_ctq, np.float32).T                        # [512, 8]
    whi = wt.astype(bf16)
    wlo = (wt - whi.astype(np.float32)).astype(bf16)
    cv, cvt = _make_consts(np.asarray(b_ctq, np.float32),
                           np.asarray(q_params, np.float32),
                           np.asarray(W_cls, np.float32),
                           np.asarray(b_cls, np.float32))
    x = np.asarray(x, np.float32)
    in_maps = []
    for c in range(NCORES):
        xs = x[c * BC:(c + 1) * BC]                             # [8192, 512]
        # relayout: [p, m*512 + k*128 + ms] = xs[128 m + ms, 128 k + p]
        xt = xs.reshape(NG, 128, NCH, 128).transpose(3, 0, 2, 1).reshape(128, BC * NCH)
        xhi = np.ascontiguousarray(xt.astype(bf16))
        xlo = np.ascontiguousarray((xt - xhi.astype(np.float32)).astype(bf16))
        in_maps.append({"xhi": xhi, "xlo": xlo, "whi": whi, "wlo": wlo,
                        "cv": cv, "cvt": cvt})
    return in_maps


def assemble_output(results):
    out = np.empty((B, 2), np.float32)
    for core in range(NCORES):
        o = results[core]["o"]                                   # [128, 2*FE]
        for c in range(2):
            # o[p, c*FE + u] = out_c(sample 128 u + p)
            out[core * BC:(core + 1) * BC, c] = \
                o[:, c * FE:(c + 1) * FE].T.reshape(BC)
    return out


def kernel(x, W_ctq, b_ctq, q_params, W_cls, b_cls):
    nc = _get_nc()
    in_maps = make_in_maps(x, W_ctq, b_ctq, q_params, W_cls, b_cls)
    res = bass_utils.run_bass_kernel_spmd(nc, in_maps, core_ids=list(range(NCORES)))
    return assemble_output(res.results)


# revision 12
# speedup vs baseline: 3.0565x; 1.1570x over previous
"""Trainium2 Bass kernel for nn_BinaryQuantumClassifier.

Math: the 4-qubit circuit collapses to a closed form. Per sample, with
theta_j = pi * (x @ W_ctq.T + b_ctq)_j  (j = 4r + i, reuse r, qubit i):
    d_i(theta) = a_i + b_i sin(theta) + c_i cos(theta)
              = a_i + R_i sin(pi * (y + b_ctq_j + phi_i/pi))
(R = hypot(b, c), phi = atan2(c, b); a/b/c derived from the fixed per-qubit
unitary RZ RY RX after RY(theta) H|0>), and the CNOT chain maps
Z-expectations to products of the d_i:
    z0 = d1 d2 d3, z1 = d0 d1, z2 = d0 d1 d2, z3 = d0 d1 d2 d3.
Output = (mean over r of z) @ W_cls.T + b_cls.

Device plan per core (8192 samples). x is relayouted on the host so the PE
uses it as the STATIONARY operand, split into bf16 hi+lo (x = xhi + xlo,
W = Whi + Wlo; 3 passes hi*hi + hi*lo + lo*hi accumulated in fp32 PSUM —
bf16 weight loads get FWL, 4x faster than fp32):
  lhsT = x-chunk [128 D x 128 samples], rhs = W-chunk [128 D x 8],
  out[128 samples, 8]; 8 sample-groups share one PSUM bank tile [128, 64].
A DVE tensor_add per bank assembles E (+ the phase-shift constant) with
free = j*32 + u, in sample-half tiles E_h [128, 256] (n = 128*(32h + u) + p).
Epilogue per half (overlaps the other half's matmuls):
  k2 = ((ysh + 1.5*2^24) - 1.5*2^24) rounds to the nearest even integer
  (exact range reduction), rsh = ysh - k2 in [-1, 1], ScalarE Sin once,
  d = a + R sin, CNOT products, mean over r, final 4->2 linear; one
  [128, 128] output tile.
"""

import numpy as np

import concourse.bass as bass
import concourse.mybir as mybir
from concourse import bass_utils
from concourse.tile import TileContext

B, D, NQ = 65536, 512, 4
NCORES = 8
BC = B // NCORES            # 8192 samples per core
NCH = D // 128              # 4 K-chunks
NS = 32                     # slabs per core
F = BC // NS                # 256 samples per slab
FE = BC // 128              # 64 samples per partition in epilogue layout
M2 = float(np.float32(1.5 * 2 ** 24))   # round-to-even-integer magic
PI = float(np.pi)
MM_DT = mybir.dt.bfloat16   # PE operand dtype: x/W split into bf16 hi+lo
                            # (3 passes hi*hi + hi*lo + lo*hi, fp32 PSUM accum)
NG = BC // 128              # 64 sample-groups per core
GPB = 8                     # groups per PSUM bank tile
AL = mybir.AluOpType
AF = mybir.ActivationFunctionType
F32 = mybir.dt.float32


def _split_waits(nc, max_waits=1):
    """walrus in this env accepts at most one sync-wait per instruction;
    move extras onto preceding same-engine NoOps."""
    for fn in nc.m.functions:
        for blk in fn.blocks:
            new_list = []
            for inst in blk.instructions:
                si = inst.sync_info
                if si is not None and len(si.on_wait) > max_waits:
                    waits = list(si.on_wait)
                    keep, extra = waits[-max_waits:], waits[:-max_waits]
                    for k, w in enumerate(extra):
                        new_list.append(mybir.InstNoOp(
                            name=f"{inst.name}-ws{k}", engine=inst.engine,
                            ins=[], outs=[],
                            sync_info=mybir.SyncInfo(on_wait=[w], on_update=[])))
                    si.on_wait = keep
                    inst.sync_info = si
                new_list.append(inst)
            blk.instructions = new_list


def _build_nc():
    nc = bass.Bass("TRN2", target_bir_lowering=False)
    # x relayout: [p, m*512 + k*128 + ms] = x[128*m + ms, 128*k + p]
    xhi_d = nc.dram_tensor("xhi", [128, BC * NCH], MM_DT, kind="ExternalInput").ap()
    xlo_d = nc.dram_tensor("xlo", [128, BC * NCH], MM_DT, kind="ExternalInput").ap()
    whi_d = nc.dram_tensor("whi", [D, 8], MM_DT, kind="ExternalInput").ap()
    wlo_d = nc.dram_tensor("wlo", [D, 8], MM_DT, kind="ExternalInput").ap()
    cv_d = nc.dram_tensor("cv", [128, 16], F32, kind="ExternalInput").ap()
    # cvt: [bsT | RT | aT], each [128, 256] laid out j*32 + u
    FH = FE // 2              # 32 sample-groups per half
    JH = 8 * FH               # 256: width of a half tile
    cvt_d = nc.dram_tensor("cvt", [128, 3 * JH], F32, kind="ExternalInput").ap()
    o_d = nc.dram_tensor("o", [128, 2 * FE], F32, kind="ExternalOutput").ap()

    with TileContext(nc) as tc:
        with tc.tile_pool(name="wp", bufs=1) as wpool, \
             tc.tile_pool(name="xp", bufs=8) as xpool, \
             tc.tile_pool(name="pp", bufs=4, space="PSUM") as pspool, \
             tc.tile_pool(name="ep", bufs=1) as epool:
            whis, wlos = [], []
            for k in range(NCH):
                whik = wpool.tile([128, 8], MM_DT, name=f"whik{k}")
                nc.sync.dma_start(whik[:], whi_d[128 * k:128 * (k + 1), :])
                whis.append(whik)
                wlok = wpool.tile([128, 8], MM_DT, name=f"wlok{k}")
                nc.sync.dma_start(wlok[:], wlo_d[128 * k:128 * (k + 1), :])
                wlos.append(wlok)
            cv = wpool.tile([128, 16], F32)
            nc.sync.dma_start(cv[:], cv_d[:])
            cvt = wpool.tile([128, 3 * JH], F32)
            nc.sync.dma_start(cvt[:], cvt_d[:])
            bsT = cvt[:, 0:JH]
            RT, aT = cvt[:, JH:2 * JH], cvt[:, 2 * JH:3 * JH]
            bs3 = bsT.rearrange("p (j u) -> p j u", j=8)

            O2 = epool.tile([128, 2 * FE], F32)
            GL = GPB * NCH * 128          # 4096: L-tile free width (8 groups)
            for h in range(2):
                # E_h: partition p = sample-in-group, free = j*FH + u
                E = epool.tile([128, JH], F32, name=f"E{h}")
                e3 = E.rearrange("p (j u) -> p j u", j=8)
                for gg in range(4):
                    g = 4 * h + gg
                    Lhi = xpool.tile([128, GL], MM_DT, tag="Lhi", name=f"Lhi{g}")
                    nc.sync.dma_start(Lhi[:], xhi_d[:, g * GL:(g + 1) * GL])
                    Llo = xpool.tile([128, GL], MM_DT, tag="Llo", name=f"Llo{g}")
                    nc.sync.dma_start(Llo[:], xlo_d[:, g * GL:(g + 1) * GL])
                    ps = pspool.tile([128, GPB * 8], F32, tag="ps", name=f"ps{g}")
                    for mm in range(GPB):
                        for k in range(NCH):
                            off = mm * (NCH * 128) + k * 128
                            out_sl = ps[:, 8 * mm:8 * mm + 8]
                            nc.tensor.matmul(out_sl, Lhi[:, off:off + 128], whis[k][:],
                                             start=(k == 0), stop=False)
                            nc.tensor.matmul(out_sl, Lhi[:, off:off + 128], wlos[k][:],
                                             start=False, stop=False)
                            nc.tensor.matmul(out_sl, Llo[:, off:off + 128], whis[k][:],
                                             start=False, stop=(k == NCH - 1))
                    # E[:, j*FH + 8gg + m] = ps[:, 8m + j] + bs (phase shift)
                    nc.vector.tensor_add(e3[:, :, GPB * gg:GPB * (gg + 1)],
                                         ps.rearrange("p (m j) -> p j m", j=8),
                                         bs3[:, :, GPB * gg:GPB * (gg + 1)])

                # ---- epilogue for this half ----
                k2 = epool.tile([128, JH], F32, name=f"k2_{h}")
                r_ = epool.tile([128, JH], F32, name=f"r_{h}")
                s_ = epool.tile([128, JH], F32, name=f"s_{h}")
                t1 = epool.tile([128, JH], F32, name=f"t1_{h}")
                d_ = epool.tile([128, JH], F32, name=f"d_{h}")
                nc.vector.tensor_scalar(k2[:], E[:], M2, M2, AL.add, AL.subtract)
                nc.vector.tensor_sub(r_[:], E[:], k2[:])       # ysh mod 2 -> [-1, 1]
                nc.scalar.activation(s_[:], r_[:], AF.Sin, scale=PI)
                nc.vector.tensor_mul(t1[:], s_[:], RT)         # R sin
                nc.vector.tensor_add(d_[:], t1[:], aT)         # d = a + R sin

                def dj(r, i):
                    j = 4 * r + i
                    return d_[:, j * FH:(j + 1) * FH]

                # products: z_k^r at Z[:, (2k + r)*FH]
                S_ = epool.tile([128, 2 * FH], F32, name=f"S_{h}")
                Z_ = epool.tile([128, 8 * FH], F32, name=f"Z_{h}")

                def zs(k, r):
                    return Z_[:, (2 * k + r) * FH:(2 * k + r + 1) * FH]

                for r in range(2):
                    u_ = S_[:, r * FH:(r + 1) * FH]
                    nc.vector.tensor_mul(u_, dj(r, 1), dj(r, 2))        # d1 d2
                    nc.vector.tensor_mul(zs(1, r), dj(r, 0), dj(r, 1))  # z1
                    nc.vector.tensor_mul(zs(2, r), dj(r, 0), u_)        # z2
                    nc.vector.tensor_mul(zs(0, r), u_, dj(r, 3))        # z0
                    nc.vector.tensor_mul(zs(3, r), zs(2, r), dj(r, 3))  # z3
                Mn = epool.tile([128, 4 * FH], F32, name=f"Mn{h}")
                for k in range(4):
                    nc.vector.tensor_add(Mn[:, k * FH:(k + 1) * FH], zs(k, 0), zs(k, 1))

                # final linear: W' = 0.5*W_cls via cv columns
                O1 = epool.tile([128, 2 * FH], F32, name=f"O1_{h}")

                def mk(k):
                    return Mn[:, k * FH:(k + 1) * FH]

                for c in range(2):
                    o1a = O1[:, c * FH:(c + 1) * FH]
                    o1b = S_[:, c * FH:(c + 1) * FH]    # reuse S_ as scratch
                    o2 = O2[:, c * FE + FH * h:c * FE + FH * (h + 1)]
                    nc.vector.tensor_scalar(o1a, mk(0), cv[:, 4 + 4 * c:5 + 4 * c],
                                            cv[:, 12 + c:13 + c], AL.mult, AL.add)
                    nc.vector.scalar_tensor_tensor(o1b, mk(1), cv[:, 5 + 4 * c:6 + 4 * c],
                                                   o1a, AL.mult, AL.add)
                    nc.vector.scalar_tensor_tensor(o1a, mk(2), cv[:, 6 + 4 * c:7 + 4 * c],
                                                   o1b, AL.mult, AL.add)
                    nc.vector.scalar_tensor_tensor(o2, mk(3), cv[:, 7 + 4 * c:8 + 4 * c],
                                                   o1a, AL.mult, AL.add)
            nc.sync.dma_start(o_d[:], O2[:])

    return nc


_NC_CACHE = {}


def _get_nc(split=True):
    key = ("nc", split)
    if key not in _NC_CACHE:
        nc = _build_nc()
        if split:
            _split_waits(nc)
        _NC_CACHE[key] = nc
    return _NC_CACHE[key]


def _qubit_abc(q_params):
    """Exact (a_i, b_i, c_i) with d_i(theta) = a + b sin(theta) + c cos(theta)."""
    out = np.zeros((NQ, 3), np.float64)
    for i in range(NQ):
        pa, pb, pc = [float(v) for v in q_params[3 * i:3 * i + 3]]

        def rx(t):
            return np.array([[np.cos(t / 2), -1j * np.sin(t / 2)],
                             [-1j * np.sin(t / 2), np.cos(t / 2)]])

        def ry(t):
            return np.array([[np.cos(t / 2), -np.sin(t / 2)],
                             [np.sin(t / 2), np.cos(t / 2)]])

        def rz(t):
            return np.array([[np.exp(-0.5j * t), 0], [0, np.exp(0.5j * t)]])

        H = np.array([[1, 1], [1, -1]]) / np.sqrt(2)
        U = rz(pc) @ ry(pb) @ rx(pa)

        def dfun(theta):
            v = U @ ry(theta) @ H @ np.array([1.0, 0.0])
            pr = np.abs(v) ** 2
            return pr[0] - pr[1]

        d0, dpi, dh = dfun(0.0), dfun(np.pi), dfun(np.pi / 2)
        a = (d0 + dpi) / 2
        c = (d0 - dpi) / 2
        b = dh - a
        out[i] = (a, b, c)
    return out


def _make_consts(b_ctq, q_params, W_cls, b_cls):
    abc = _qubit_abc(q_params)
    cv = np.zeros((128, 16), np.float32)
    wp = 0.5 * np.asarray(W_cls, np.float64)      # mean over r folded in
    for c in range(2):
        for k in range(4):
            cv[:, 4 + 4 * c + k] = np.float32(wp[c, k])
        cv[:, 12 + c] = np.float32(b_cls[c])
    FH = FE // 2
    JH = 8 * FH
    cvt = np.zeros((128, 3 * JH), np.float32)
    for j in range(8):
        i = j % 4
        a, b, c_ = abc[i]
        R = np.hypot(b, c_)
        phi = np.arctan2(c_, b)
        cvt[:, 0 * JH + j * FH:0 * JH + (j + 1) * FH] = np.float32(b_ctq[j] + phi / np.pi)
        cvt[:, 1 * JH + j * FH:1 * JH + (j + 1) * FH] = np.float32(R)
        cvt[:, 2 * JH + j * FH:2 * JH + (j + 1) * FH] = np.float32(a)
    return cv, cvt


def make_in_maps(x, W_ctq, b_ctq, q_params, W_cls, b_cls):
    import ml_dtypes
    bf16 = ml_dtypes.bfloat16
    wt = np.asarray(W_ctq, np.float32).T                        # [512, 8]
    whi = wt.astype(bf16)
    wlo = (wt - whi.astype(np.float32)).astype(bf16)
    cv, cvt = _make_consts(np.asarray(b_ctq, np.float32),
                           np.asarray(q_params, np.float32),
                           np.asarray(W_cls, np.float32),
                           np.asarray(b_cls, np.float32))
    x = np.asarray(x, np.float32)
    in_maps = []
    for c in range(NCORES):
        xs = x[c * BC:(c + 1) * BC]                             # [8192, 512]
        # relayout: [p, m*512 + k*128 + ms] = xs[128 m + ms, 128 k + p]
        xt = xs.reshape(NG, 128, NCH, 128).transpose(3, 0, 2, 1).reshape(128, BC * NCH)
        xhi = np.ascontiguousarray(xt.astype(bf16))
        xlo = np.ascontiguousarray((xt - xhi.astype(np.float32)).astype(bf16))
        in_maps.append({"xhi": xhi, "xlo": xlo, "whi": whi, "wlo": wlo,
                        "cv": cv, "cvt": cvt})
    return in_maps


def assemble_output(results):
    out = np.empty((B, 2), np.float32)
    for core in range(NCORES):
        o = results[core]["o"]                                   # [128, 2*FE]
        for c in range(2):
            # o[p, c*FE + u] = out_c(sample 128 u + p)
            out[core * BC:(core + 1) * BC, c] = \
                o[:, c * FE:(c + 1) * FE].T.reshape(BC)
    return out


def kernel(x, W_ctq, b_ctq, q_params, W_cls, b_cls):
    nc = _get_nc()
    in_maps = make_in_maps(x, W_ctq, b_ctq, q_params, W_cls, b_cls)
    res = bass_utils.run_bass_kernel_spmd(nc, in_maps, core_ids=list(range(NCORES)))
    return assemble_output(res.results)
